# revision 1
# baseline (speedup 1.0000x reference)
"""GCMC graph-conv kernel for Trainium2, distributed over 8 NeuronCores.

Computes: agg = segment_sum((src_feats @ W.T + b) * cj [edge_src], edge_dst) * ci

Strategy (dst-sharded, one NEFF SPMD on 8 cores):
  - Each core owns 12500 destination nodes and the edges pointing to them.
  - Phase A: each core computes wh = (X_shard @ W.T + b) * cj_shard on the
    TensorEngine and writes it (bf16) into a packed table shard: each 256B row
    holds FOUR nodes' 32-feature messages (node prow -> row prow//4, subcol
    prow%4). Packing keeps dma_gather rows at the required 256B multiple while
    the whole 25088-row table stays addressable by int16 gather indices.
  - AllGather the 8 compact shards (0.8MB each) -> full table in every HBM.
  - Phase B: edges are bucketed by (dst block, q=prow%4, dst half). Each
    128-slot tile gathers its edges' table rows (dma_gather), builds a
    one-hot matrix over its 64-dst window (is_equal on VectorE), and
    scatter-sums via PSUM-accumulating matmuls (rhs = gathered columns
    [32q : 32q+32]). Scale by ci, DMA out.

All control structure (tile counts, windows) is common across the 8 cores
(max over cores); cores pad their slots (dst_shift=-1 kills the one-hot
column; gather idx 0 is harmless).
"""
import sys

if "/opt/trn_rl_repo" not in sys.path:
    sys.path.insert(0, "/opt/trn_rl_repo")

import numpy as np
import ml_dtypes

import concourse.bacc as bacc
import concourse.mybir as mybir
import concourse.tile as tile
from concourse.bass_utils import run_bass_kernel_spmd

# problem constants (hardcoded per harness contract)
N_NODES = 100000
N_EDGES = 1_600_000
IN_DIM = 128
OUT_DIM = 32
N_CORES = 8
SHARD = N_NODES // N_CORES          # 12500 dst nodes per core
NBLK = (SHARD + 127) // 128         # 98 dst blocks per core
SPAD = NBLK * 128                   # 12544 padded shard nodes
TROWS = SPAD * N_CORES // 4         # 25088 packed table rows (4 nodes each)
LROWS = SPAD // 4                   # 3136 packed rows per core shard
ROWELEM = 128                       # bf16 elems per table row = 256B
WIN = 128                           # one-hot window: full block (PSUM base 0)
GRP = 8                             # tiles per is_equal op
BB = 5                              # dst blocks per double-buffered batch
GCAP = 25                           # tiles per dma_gather call

F32 = mybir.dt.float32
BF16 = mybir.dt.bfloat16
I16 = mybir.dt.int16


def _plan(edge_src, edge_dst):
    """Pack edges into the common SPMD structure.

    meta:
      ntiles       total tiles
      lo_of[t]     PSUM window base (0 or 64)
      q_of[t]      table subcolumn (edge prow % 4)
      toff[b]      first tile of dst block b
      batches      list of (b0, b1)
    per core:
      idx  [128, ntiles*8] int16  wrapped packed-row gather indices
      dst  [128, ntiles]   bf16   per-slot dst_shift in window (-1 = pad)
    """
    src = np.asarray(edge_src).astype(np.int64)
    dst = np.asarray(edge_dst).astype(np.int64)

    core = dst // SHARD
    dst_loc = dst % SHARD
    blk = dst_loc // 128
    dib = dst_loc % 128
    prow = (src // SHARD) * SPAD + (src % SHARD)
    row = prow // 4
    q = prow % 4

    key = ((core * NBLK + blk) * 4 + q)
    order = np.argsort(key, kind="stable")
    s_key, s_dib, s_row = key[order], dib[order], row[order]

    n_cells = N_CORES * NBLK * 4
    bounds = np.searchsorted(s_key, np.arange(n_cells + 1))

    ntiles = 0
    lo_of, q_of = [], []
    toff = np.zeros(NBLK + 1, np.int64)
    idx_cols = [[] for _ in range(N_CORES)]
    sh_cols = [[] for _ in range(N_CORES)]

    for b in range(NBLK):
        for kq in range(4):
            for h in range(1):
                segs = []
                for c in range(N_CORES):
                    cid = (c * NBLK + b) * 4 + kq
                    segs.append((int(bounds[cid]), int(bounds[cid + 1])))
                nt = (max(e - s for s, e in segs) + 127) // 128
                lo = 0
                for t in range(nt):
                    ntiles += 1
                    lo_of.append(lo)
                    q_of.append(kq)
                    for c in range(N_CORES):
                        s, e = segs[c]
                        p = s + t * 128
                        take = max(0, min(e - p, 128))
                        col_i = np.zeros(128, np.int16)
                        col_s = np.full(128, -1.0, np.float32)
                        if take > 0:
                            col_i[:take] = s_row[p:p + take]
                            col_s[:take] = s_dib[p:p + take] - lo
                        idx_cols[c].append(col_i)
                        sh_cols[c].append(col_s)
        toff[b + 1] = ntiles

    batches = [(b0, min(b0 + BB, NBLK)) for b0 in range(0, NBLK, BB)]
    meta = {"ntiles": ntiles, "lo_of": lo_of, "q_of": q_of, "toff": toff,
            "batches": batches}

    per_core = []
    for c in range(N_CORES):
        icols = np.stack(idx_cols[c], 0)          # [nt, 128]
        scols = np.stack(sh_cols[c], 0)           # [nt, 128]
        w = icols.reshape(ntiles, 8, 16).transpose(2, 0, 1).reshape(16, ntiles * 8)
        per_core.append({
            "idx": np.tile(w.astype(np.int16), (8, 1)),
            "dst": scols.T.astype(ml_dtypes.bfloat16),
        })
    return meta, per_core


def _phasea_perm():
    """Phase-A node processing order: tile t covers packed rows [32t, 32t+32);
    partition p holds local node 4*(32t + p%32) + p//32."""
    t = np.arange(SPAD) // 128
    p = np.arange(SPAD) % 128
    return 4 * (32 * t + p % 32) + p // 32


def _build(meta, mode="full", n_devices=N_CORES, no_cc=False, reps=1):
    ntiles = meta["ntiles"]
    lo_of = meta["lo_of"]
    q_of = meta["q_of"]
    toff = meta["toff"]
    batches = meta["batches"]

    nc = bacc.Bacc("TRN2", target_bir_lowering=False, debug=False,
                   enable_asserts=True, num_devices=n_devices)

    xT = nc.dram_tensor("xT", [128, SPAD], F32, kind="ExternalInput")
    wT = nc.dram_tensor("wT", [128, OUT_DIM], F32, kind="ExternalInput")
    brep = nc.dram_tensor("brep", [128, OUT_DIM], F32, kind="ExternalInput")
    cjT = nc.dram_tensor("cjT", [128, NBLK], F32, kind="ExternalInput")
    ciT = nc.dram_tensor("ciT", [128, NBLK], F32, kind="ExternalInput")
    idx_d = nc.dram_tensor("idx", [128, ntiles * 8], I16, kind="ExternalInput")
    dst_d = nc.dram_tensor("dst", [128, ntiles], BF16, kind="ExternalInput")
    out = nc.dram_tensor("out", [SPAD, OUT_DIM], F32, kind="ExternalOutput")

    gmax = 1
    for (b0, b1) in batches:
        gmax = max(gmax, int(toff[b1] - toff[b0]))

    with tile.TileContext(nc) as tc:
        with (
            tc.tile_pool(name="dram", bufs=1, space="DRAM") as dram,
            tc.tile_pool(name="const", bufs=1) as cpool,
            tc.tile_pool(name="xa", bufs=3) as xpool,
            tc.tile_pool(name="ha", bufs=3) as hpool,
            tc.tile_pool(name="wa", bufs=3) as wpool,
            tc.tile_pool(name="pa", bufs=4, space="PSUM") as ppa,
            tc.tile_pool(name="gath", bufs=2) as gpool,
            tc.tile_pool(name="smat", bufs=2) as spool,
            tc.tile_pool(name="pb", bufs=4, space="PSUM") as ppb,
            tc.tile_pool(name="res", bufs=4) as rpool,
        ):
            table_loc = dram.tile([LROWS, ROWELEM], BF16)
            table_full = dram.tile([TROWS, ROWELEM], BF16)

            # constants
            wt_t = cpool.tile([128, OUT_DIM], F32)
            nc.sync.dma_start(out=wt_t[:], in_=wT[:])
            br_t = cpool.tile([128, OUT_DIM], F32)
            nc.sync.dma_start(out=br_t[:], in_=brep[:])
            cj_t = cpool.tile([128, NBLK], F32)
            nc.sync.dma_start(out=cj_t[:], in_=cjT[:])
            ci_t = cpool.tile([128, NBLK], F32)
            nc.sync.dma_start(out=ci_t[:], in_=ciT[:])
            idx_t = cpool.tile([128, ntiles * 8], I16)
            nc.sync.dma_start(out=idx_t[:], in_=idx_d[:])
            dst_t = cpool.tile([128, ntiles], BF16)
            nc.sync.dma_start(out=dst_t[:], in_=dst_d[:])
            # iota: [128, GRP*WIN] bf16, value = col % WIN
            io_i = cpool.tile([128, GRP * WIN], I16)
            nc.gpsimd.iota(io_i[:], pattern=[[0, GRP], [1, WIN]], base=0,
                           channel_multiplier=0)
            io_b = cpool.tile([128, GRP * WIN], BF16)
            nc.vector.tensor_copy(out=io_b[:], in_=io_i[:])
            z128 = cpool.tile([128, 128], BF16)
            nc.vector.memset(z128[:], 0)
            z32 = cpool.tile([128, OUT_DIM], BF16)
            nc.vector.memset(z32[:], 0)

            # packed-table write AP: (q, r, t, f) view of [LROWS, 128]
            tab_v = table_loc[:].rearrange("(t r) (q f) -> q r t f", r=32, q=4)

            for _rep in range(reps):
                # ---- Phase A: wh = (X @ W.T + b) * cj -> packed bf16 shard ----
                ntile_a = SPAD // 128  # 98
                for a0 in range(0, ntile_a, 4):
                    an = min(4, ntile_a - a0)
                    xt = xpool.tile([128, 4 * 128], F32)
                    nc.sync.dma_start(out=xt[:, 0:an * 128],
                                      in_=xT[:, a0 * 128:(a0 + an) * 128])
                    wh4 = wpool.tile([128, 4, OUT_DIM], BF16)
                    for j in range(an):
                        ph = ppa.tile([128, OUT_DIM], F32, space="PSUM")
                        nc.tensor.matmul(out=ph[:], lhsT=xt[:, j * 128:(j + 1) * 128],
                                         rhs=wt_t[:], start=True, stop=True)
                        hb = hpool.tile([128, OUT_DIM], F32)
                        nc.vector.tensor_add(out=hb[:], in0=ph[:], in1=br_t[:])
                        nc.vector.tensor_scalar_mul(wh4[:, j, :], hb[:],
                                                    cj_t[:, a0 + j:a0 + j + 1])
                    for j in range(an):
                        nc.sync.dma_start(out=tab_v[:, :, a0 + j, :],
                                          in_=wh4[:, j, :])

                # ---- AllGather compact table shards ----
                if mode != "A" and not no_cc:
                    nc.gpsimd.collective_compute(
                        "AllGather",
                        mybir.AluOpType.bypass,
                        replica_groups=[list(range(N_CORES))],
                        ins=[table_loc.opt()],
                        outs=[table_full.opt()],
                    )

                # ---- Phase B ----
                for (b0, b1) in batches:
                    t0, t1 = int(toff[b0]), int(toff[b1])
                    tcnt = t1 - t0
                    g = gpool.tile([128, gmax, ROWELEM], BF16, tag="g")
                    s = spool.tile([128, gmax * WIN], BF16, tag="s")
                    if tcnt > 0 and mode not in ("A", "AG"):
                        for c0 in range(0, tcnt, GCAP):
                            cn = min(GCAP, tcnt - c0)
                            nc.gpsimd.dma_gather(
                                out_ap=g[:, c0:c0 + cn, :],
                                in_ap=table_full[:],
                                idxs_ap=idx_t[:, (t0 + c0) * 8:(t0 + c0 + cn) * 8],
                                num_idxs=cn * 128,
                                num_idxs_reg=cn * 128,
                                elem_size=ROWELEM,
                                single_packet=False,
                            )
                        if mode != "G":
                            for g0 in range(0, tcnt, GRP):
                                cnt = min(GRP, tcnt - g0)
                                nc.vector.tensor_tensor(
                                    out=s[:, g0 * WIN:(g0 + cnt) * WIN],
                                    in0=dst_t[:, t0 + g0:t0 + g0 + cnt, None]
                                        .to_broadcast([128, cnt, WIN]),
                                    in1=io_b[:, 0:cnt * WIN],
                                    op=mybir.AluOpType.is_equal,
                                )

                    for b in range(b0, b1):
                        acc = ppb.tile([128, OUT_DIM], F32, space="PSUM")
                        nc.tensor.matmul(out=acc[:], lhsT=z128[:], rhs=z32[:],
                                         start=True, stop=(mode != "full"),
                                         skip_group_check=True)
                        if mode == "full":
                            tb0, tb1 = int(toff[b]), int(toff[b + 1])
                            for t in range(tb0, tb1):
                                gi = t - t0
                                lo = lo_of[t]
                                kq = q_of[t]
                                nc.tensor.matmul(
                                    out=acc[lo:lo + WIN, :],
                                    lhsT=s[:, gi * WIN:(gi + 1) * WIN],
                                    rhs=g[:, gi, 32 * kq:32 * kq + OUT_DIM],
                                    start=False, stop=(t == tb1 - 1),
                                    skip_group_check=True,
                                )
                        res = rpool.tile([128, OUT_DIM], F32)
                        nc.vector.tensor_scalar_mul(res[:], acc[:], ci_t[:, b:b + 1])
                        nc.sync.dma_start(out=out[b * 128:(b + 1) * 128, :], in_=res[:])
    nc.compile()
    return nc


def _in_maps(ins, per_core):
    src_feats = np.ascontiguousarray(np.asarray(ins["src_feats"], dtype=np.float32))
    cj = np.asarray(ins["cj"], dtype=np.float32).reshape(-1)
    ci = np.asarray(ins["ci"], dtype=np.float32).reshape(-1)
    W = np.asarray(ins["W"], dtype=np.float32)
    b = np.asarray(ins["b"], dtype=np.float32).reshape(-1)

    perm = _phasea_perm()
    maps = []
    for c in range(N_CORES):
        lo, hi = c * SHARD, (c + 1) * SHARD
        xf = np.zeros((SPAD, IN_DIM), np.float32)
        xf[:SHARD] = src_feats[lo:hi]
        cjf = np.zeros(SPAD, np.float32)
        cjf[:SHARD] = cj[lo:hi]
        cif = np.zeros(SPAD, np.float32)
        cif[:SHARD] = ci[lo:hi]
        xP = xf[perm]            # phase-A processing order
        cjP = cjf[perm]
        m = {
            "xT": np.ascontiguousarray(xP.T),
            "wT": np.ascontiguousarray(W.T),
            "brep": np.tile(b[None, :], (128, 1)),
            "cjT": np.ascontiguousarray(cjP.reshape(NBLK, 128).T),
            "ciT": np.ascontiguousarray(cif.reshape(NBLK, 128).T),
        }
        m.update(per_core[c])
        maps.append(m)
    return maps


def kernel(src_feats, cj, ci, W, b, edge_src, edge_dst):
    ins = {"src_feats": src_feats, "cj": cj, "ci": ci, "W": W, "b": b}
    meta, per_core = _plan(edge_src, edge_dst)
    nc = _build(meta)
    maps = _in_maps(ins, per_core)
    res = run_bass_kernel_spmd(nc, maps, core_ids=list(range(N_CORES)))
    outs = [res.results[c]["out"][:SHARD] for c in range(N_CORES)]
    return np.concatenate(outs, 0).astype(np.float32)



# revision 3
# speedup vs baseline: 1.2928x; 1.2928x over previous
"""GCMC graph-conv kernel for Trainium2, distributed over 8 NeuronCores.

Computes: agg = segment_sum((src_feats @ W.T + b) * cj [edge_src], edge_dst) * ci

Strategy (dst-sharded, one NEFF SPMD on 8 cores):
  - Each core owns 12500 destination nodes and the edges pointing to them.
  - Phase A: each core computes wh = (X_shard @ W.T + b) * cj_shard on the
    TensorEngine and writes it (bf16) into a packed table shard: each 256B row
    holds FOUR nodes' 32-feature messages (node prow -> row prow//4, subcol
    prow%4). Packing keeps dma_gather rows at the required 256B multiple while
    the whole 25088-row table stays addressable by int16 gather indices.
  - AllGather the 8 compact shards (0.8MB each) -> full table in every HBM.
  - Phase B: edges are bucketed by (dst block, q=prow%4, dst half). Each
    128-slot tile gathers its edges' table rows (dma_gather), builds a
    one-hot matrix over its 64-dst window (is_equal on VectorE), and
    scatter-sums via PSUM-accumulating matmuls (rhs = gathered columns
    [32q : 32q+32]). Scale by ci, DMA out.

All control structure (tile counts, windows) is common across the 8 cores
(max over cores); cores pad their slots (dst_shift=-1 kills the one-hot
column; gather idx 0 is harmless).
"""
import sys

if "/opt/trn_rl_repo" not in sys.path:
    sys.path.insert(0, "/opt/trn_rl_repo")

import numpy as np
import ml_dtypes

import concourse.bacc as bacc
import concourse.mybir as mybir
import concourse.tile as tile
from concourse.bass_utils import run_bass_kernel_spmd

# problem constants (hardcoded per harness contract)
N_NODES = 100000
N_EDGES = 1_600_000
IN_DIM = 128
OUT_DIM = 32
N_CORES = 8
SHARD = N_NODES // N_CORES          # 12500 dst nodes per core
NBLK = (SHARD + 127) // 128         # 98 dst blocks per core
SPAD = NBLK * 128                   # 12544 padded shard nodes
TROWS = SPAD * N_CORES // 4         # 25088 packed table rows (4 nodes each)
LROWS = SPAD // 4                   # 3136 packed rows per core shard
ROWELEM = 128                       # bf16 elems per table row = 256B
WIN = 128                           # one-hot window: full block (PSUM base 0)
GRP = 8                             # tiles per is_equal op
BB = 5                              # dst blocks per double-buffered batch
GCAP = 25                           # tiles per dma_gather call

F32 = mybir.dt.float32
BF16 = mybir.dt.bfloat16
I16 = mybir.dt.int16


def _plan(edge_src, edge_dst):
    """Pack edges into the common SPMD structure.

    meta:
      ntiles       total tiles
      lo_of[t]     PSUM window base (0 or 64)
      q_of[t]      table subcolumn (edge prow % 4)
      toff[b]      first tile of dst block b
      batches      list of (b0, b1)
    per core:
      idx  [128, ntiles*8] int16  wrapped packed-row gather indices
      dst  [128, ntiles]   bf16   per-slot dst_shift in window (-1 = pad)
    """
    src = np.asarray(edge_src).astype(np.int64)
    dst = np.asarray(edge_dst).astype(np.int64)

    core = dst // SHARD
    dst_loc = dst % SHARD
    blk = dst_loc // 128
    dib = dst_loc % 128
    prow = (src // SHARD) * SPAD + (src % SHARD)
    row = prow // 4
    q = prow % 4

    key = ((core * NBLK + blk) * 4 + q)
    order = np.argsort(key, kind="stable")
    s_key, s_dib, s_row = key[order], dib[order], row[order]

    n_cells = N_CORES * NBLK * 4
    bounds = np.searchsorted(s_key, np.arange(n_cells + 1))

    ntiles = 0
    lo_of, q_of = [], []
    toff = np.zeros(NBLK + 1, np.int64)
    idx_cols = [[] for _ in range(N_CORES)]
    sh_cols = [[] for _ in range(N_CORES)]

    for b in range(NBLK):
        for kq in range(4):
            for h in range(1):
                segs = []
                for c in range(N_CORES):
                    cid = (c * NBLK + b) * 4 + kq
                    segs.append((int(bounds[cid]), int(bounds[cid + 1])))
                nt = (max(e - s for s, e in segs) + 127) // 128
                lo = 0
                for t in range(nt):
                    ntiles += 1
                    lo_of.append(lo)
                    q_of.append(kq)
                    for c in range(N_CORES):
                        s, e = segs[c]
                        p = s + t * 128
                        take = max(0, min(e - p, 128))
                        col_i = np.zeros(128, np.int16)
                        col_s = np.full(128, -1.0, np.float32)
                        if take > 0:
                            col_i[:take] = s_row[p:p + take]
                            col_s[:take] = s_dib[p:p + take] - lo
                        idx_cols[c].append(col_i)
                        sh_cols[c].append(col_s)
        toff[b + 1] = ntiles

    batches = [(b0, min(b0 + BB, NBLK)) for b0 in range(0, NBLK, BB)]
    meta = {"ntiles": ntiles, "lo_of": lo_of, "q_of": q_of, "toff": toff,
            "batches": batches}

    per_core = []
    for c in range(N_CORES):
        icols = np.stack(idx_cols[c], 0)          # [nt, 128]
        scols = np.stack(sh_cols[c], 0)           # [nt, 128]
        w = icols.reshape(ntiles, 8, 16).transpose(2, 0, 1).reshape(16, ntiles * 8)
        per_core.append({
            "idx": np.tile(w.astype(np.int16), (8, 1)),
            "dst": scols.T.astype(ml_dtypes.bfloat16),
        })
    return meta, per_core


def _phasea_perm():
    """Phase-A node processing order: tile t covers packed rows [32t, 32t+32);
    partition p holds local node 4*(32t + p%32) + p//32."""
    t = np.arange(SPAD) // 128
    p = np.arange(SPAD) % 128
    return 4 * (32 * t + p % 32) + p // 32


def _build(meta, mode="full", n_devices=N_CORES, no_cc=False, reps=1):
    ntiles = meta["ntiles"]
    lo_of = meta["lo_of"]
    q_of = meta["q_of"]
    toff = meta["toff"]
    batches = meta["batches"]

    nc = bacc.Bacc("TRN2", target_bir_lowering=False, debug=False,
                   enable_asserts=True, num_devices=n_devices,
                   num_swdge_queues=4)

    xT = nc.dram_tensor("xT", [128, SPAD], F32, kind="ExternalInput")
    wT = nc.dram_tensor("wT", [128, OUT_DIM], F32, kind="ExternalInput")
    brep = nc.dram_tensor("brep", [128, OUT_DIM], F32, kind="ExternalInput")
    cjT = nc.dram_tensor("cjT", [128, NBLK], F32, kind="ExternalInput")
    ciT = nc.dram_tensor("ciT", [128, NBLK], F32, kind="ExternalInput")
    idx_d = nc.dram_tensor("idx", [128, ntiles * 8], I16, kind="ExternalInput")
    dst_d = nc.dram_tensor("dst", [128, ntiles], BF16, kind="ExternalInput")
    out = nc.dram_tensor("out", [SPAD, OUT_DIM], F32, kind="ExternalOutput")

    gmax = 1
    for (b0, b1) in batches:
        gmax = max(gmax, int(toff[b1] - toff[b0]))

    with tile.TileContext(nc) as tc:
        with (
            tc.tile_pool(name="dram", bufs=1, space="DRAM") as dram,
            tc.tile_pool(name="const", bufs=1) as cpool,
            tc.tile_pool(name="xa", bufs=3) as xpool,
            tc.tile_pool(name="ha", bufs=3) as hpool,
            tc.tile_pool(name="wa", bufs=3) as wpool,
            tc.tile_pool(name="pa", bufs=4, space="PSUM") as ppa,
            tc.tile_pool(name="gath", bufs=2) as gpool,
            tc.tile_pool(name="smat", bufs=2) as spool,
            tc.tile_pool(name="pb", bufs=4, space="PSUM") as ppb,
            tc.tile_pool(name="res", bufs=4) as rpool,
        ):
            table_loc = dram.tile([LROWS, ROWELEM], BF16)
            table_full = dram.tile([TROWS, ROWELEM], BF16)

            # constants
            wt_t = cpool.tile([128, OUT_DIM], F32)
            nc.sync.dma_start(out=wt_t[:], in_=wT[:])
            br_t = cpool.tile([128, OUT_DIM], F32)
            nc.sync.dma_start(out=br_t[:], in_=brep[:])
            cj_t = cpool.tile([128, NBLK], F32)
            nc.sync.dma_start(out=cj_t[:], in_=cjT[:])
            ci_t = cpool.tile([128, NBLK], F32)
            nc.sync.dma_start(out=ci_t[:], in_=ciT[:])
            idx_t = cpool.tile([128, ntiles * 8], I16)
            nc.sync.dma_start(out=idx_t[:], in_=idx_d[:])
            dst_t = cpool.tile([128, ntiles], BF16)
            nc.sync.dma_start(out=dst_t[:], in_=dst_d[:])
            # iota: [128, GRP*WIN] bf16, value = col % WIN
            io_i = cpool.tile([128, GRP * WIN], I16)
            nc.gpsimd.iota(io_i[:], pattern=[[0, GRP], [1, WIN]], base=0,
                           channel_multiplier=0)
            io_b = cpool.tile([128, GRP * WIN], BF16)
            nc.vector.tensor_copy(out=io_b[:], in_=io_i[:])
            z128 = cpool.tile([128, 128], BF16)
            nc.vector.memset(z128[:], 0)
            z32 = cpool.tile([128, OUT_DIM], BF16)
            nc.vector.memset(z32[:], 0)

            # packed-table write AP: (q, r, t, f) view of [LROWS, 128]
            tab_v = table_loc[:].rearrange("(t r) (q f) -> q r t f", r=32, q=4)

            for _rep in range(reps):
                # ---- Phase A: wh = (X @ W.T + b) * cj -> packed bf16 shard ----
                ntile_a = SPAD // 128  # 98
                for a0 in range(0, ntile_a, 4):
                    an = min(4, ntile_a - a0)
                    xt = xpool.tile([128, 4 * 128], F32)
                    nc.sync.dma_start(out=xt[:, 0:an * 128],
                                      in_=xT[:, a0 * 128:(a0 + an) * 128])
                    wh4 = wpool.tile([128, 4, OUT_DIM], BF16)
                    for j in range(an):
                        ph = ppa.tile([128, OUT_DIM], F32, space="PSUM")
                        nc.tensor.matmul(out=ph[:], lhsT=xt[:, j * 128:(j + 1) * 128],
                                         rhs=wt_t[:], start=True, stop=True)
                        hb = hpool.tile([128, OUT_DIM], F32)
                        nc.vector.tensor_add(out=hb[:], in0=ph[:], in1=br_t[:])
                        nc.vector.tensor_scalar_mul(wh4[:, j, :], hb[:],
                                                    cj_t[:, a0 + j:a0 + j + 1])
                    for j in range(an):
                        nc.sync.dma_start(out=tab_v[:, :, a0 + j, :],
                                          in_=wh4[:, j, :])

                # ---- AllGather compact table shards ----
                if mode != "A" and not no_cc:
                    nc.gpsimd.collective_compute(
                        "AllGather",
                        mybir.AluOpType.bypass,
                        replica_groups=[list(range(N_CORES))],
                        ins=[table_loc.opt()],
                        outs=[table_full.opt()],
                    )

                # ---- Phase B ----
                for (b0, b1) in batches:
                    t0, t1 = int(toff[b0]), int(toff[b1])
                    tcnt = t1 - t0
                    g = gpool.tile([128, gmax, ROWELEM], BF16, tag="g")
                    s = spool.tile([128, gmax * WIN], BF16, tag="s")
                    if tcnt > 0 and mode not in ("A", "AG"):
                        for c0 in range(0, tcnt, GCAP):
                            cn = min(GCAP, tcnt - c0)
                            nc.gpsimd.dma_gather(
                                out_ap=g[:, c0:c0 + cn, :],
                                in_ap=table_full[:],
                                idxs_ap=idx_t[:, (t0 + c0) * 8:(t0 + c0 + cn) * 8],
                                num_idxs=cn * 128,
                                num_idxs_reg=cn * 128,
                                elem_size=ROWELEM,
                                single_packet=False,
                                queue_num=(c0 // GCAP) % 4,
                            )
                        if mode != "G":
                            for g0 in range(0, tcnt, GRP):
                                cnt = min(GRP, tcnt - g0)
                                nc.vector.tensor_tensor(
                                    out=s[:, g0 * WIN:(g0 + cnt) * WIN],
                                    in0=dst_t[:, t0 + g0:t0 + g0 + cnt, None]
                                        .to_broadcast([128, cnt, WIN]),
                                    in1=io_b[:, 0:cnt * WIN],
                                    op=mybir.AluOpType.is_equal,
                                )

                    for b in range(b0, b1):
                        acc = ppb.tile([128, OUT_DIM], F32, space="PSUM")
                        nc.tensor.matmul(out=acc[:], lhsT=z128[:], rhs=z32[:],
                                         start=True, stop=(mode != "full"),
                                         skip_group_check=True)
                        if mode == "full":
                            tb0, tb1 = int(toff[b]), int(toff[b + 1])
                            for t in range(tb0, tb1):
                                gi = t - t0
                                lo = lo_of[t]
                                kq = q_of[t]
                                nc.tensor.matmul(
                                    out=acc[lo:lo + WIN, :],
                                    lhsT=s[:, gi * WIN:(gi + 1) * WIN],
                                    rhs=g[:, gi, 32 * kq:32 * kq + OUT_DIM],
                                    start=False, stop=(t == tb1 - 1),
                                    skip_group_check=True,
                                )
                        res = rpool.tile([128, OUT_DIM], F32)
                        nc.vector.tensor_scalar_mul(res[:], acc[:], ci_t[:, b:b + 1])
                        nc.sync.dma_start(out=out[b * 128:(b + 1) * 128, :], in_=res[:])
    nc.compile()
    return nc


def _in_maps(ins, per_core):
    src_feats = np.ascontiguousarray(np.asarray(ins["src_feats"], dtype=np.float32))
    cj = np.asarray(ins["cj"], dtype=np.float32).reshape(-1)
    ci = np.asarray(ins["ci"], dtype=np.float32).reshape(-1)
    W = np.asarray(ins["W"], dtype=np.float32)
    b = np.asarray(ins["b"], dtype=np.float32).reshape(-1)

    perm = _phasea_perm()
    maps = []
    for c in range(N_CORES):
        lo, hi = c * SHARD, (c + 1) * SHARD
        xf = np.zeros((SPAD, IN_DIM), np.float32)
        xf[:SHARD] = src_feats[lo:hi]
        cjf = np.zeros(SPAD, np.float32)
        cjf[:SHARD] = cj[lo:hi]
        cif = np.zeros(SPAD, np.float32)
        cif[:SHARD] = ci[lo:hi]
        xP = xf[perm]            # phase-A processing order
        cjP = cjf[perm]
        m = {
            "xT": np.ascontiguousarray(xP.T),
            "wT": np.ascontiguousarray(W.T),
            "brep": np.tile(b[None, :], (128, 1)),
            "cjT": np.ascontiguousarray(cjP.reshape(NBLK, 128).T),
            "ciT": np.ascontiguousarray(cif.reshape(NBLK, 128).T),
        }
        m.update(per_core[c])
        maps.append(m)
    return maps


def kernel(src_feats, cj, ci, W, b, edge_src, edge_dst):
    ins = {"src_feats": src_feats, "cj": cj, "ci": ci, "W": W, "b": b}
    meta, per_core = _plan(edge_src, edge_dst)
    nc = _build(meta)
    maps = _in_maps(ins, per_core)
    res = run_bass_kernel_spmd(nc, maps, core_ids=list(range(N_CORES)))
    outs = [res.results[c]["out"][:SHARD] for c in range(N_CORES)]
    return np.concatenate(outs, 0).astype(np.float32)



# revision 5
# speedup vs baseline: 3.8018x; 2.9407x over previous
"""GCMC graph-conv kernel for Trainium2, distributed over 8 NeuronCores.

Computes: agg = segment_sum((src_feats @ W.T + b) * cj [edge_src], edge_dst) * ci

Strategy (dst-sharded, one NEFF SPMD on 8 cores):
  - Each core owns 12500 destination nodes and the edges pointing to them.
  - Phase A: each core computes wh = (X_shard @ W.T + b) * cj_shard on the
    TensorEngine and writes it (bf16) into a packed table shard: each 256B row
    holds FOUR nodes' 32-feature messages (node prow -> row prow//4, subcol
    prow%4). Packing keeps dma_gather rows at the required 256B multiple while
    the whole 25088-row table stays addressable by int16 gather indices.
  - AllGather the 8 compact shards (0.8MB each) -> full table in every HBM.
  - Phase B: edges are bucketed by (dst block, q=prow%4, dst half). Each
    128-slot tile gathers its edges' table rows (dma_gather), builds a
    one-hot matrix over its 64-dst window (is_equal on VectorE), and
    scatter-sums via PSUM-accumulating matmuls (rhs = gathered columns
    [32q : 32q+32]). Scale by ci, DMA out.

All control structure (tile counts, windows) is common across the 8 cores
(max over cores); cores pad their slots (dst_shift=-1 kills the one-hot
column; gather idx 0 is harmless).
"""
import sys

if "/opt/trn_rl_repo" not in sys.path:
    sys.path.insert(0, "/opt/trn_rl_repo")

import numpy as np
import ml_dtypes

import concourse.bacc as bacc
import concourse.mybir as mybir
import concourse.tile as tile
from concourse.bass_utils import run_bass_kernel_spmd

# problem constants (hardcoded per harness contract)
N_NODES = 100000
N_EDGES = 1_600_000
IN_DIM = 128
OUT_DIM = 32
N_CORES = 8
SHARD = N_NODES // N_CORES          # 12500 dst nodes per core
NBLK = (SHARD + 127) // 128         # 98 dst blocks per core
SPAD = NBLK * 128                   # 12544 padded shard nodes
TROWS = SPAD * N_CORES // 4         # 25088 packed table rows (4 nodes each)
LROWS = SPAD // 4                   # 3136 packed rows per core shard
ROWELEM = 128                       # bf16 elems per table row = 256B
WIN = 128                           # one-hot window: full block (PSUM base 0)
GRP = 8                             # tiles per is_equal op
BB = 5                              # dst blocks per double-buffered batch
GCAP = 25                           # tiles per dma_gather call

F32 = mybir.dt.float32
BF16 = mybir.dt.bfloat16
I16 = mybir.dt.int16


def _plan(edge_src, edge_dst):
    """Pack edges into the common SPMD structure.

    meta:
      ntiles       total tiles
      lo_of[t]     PSUM window base (0 or 64)
      q_of[t]      table subcolumn (edge prow % 4)
      toff[b]      first tile of dst block b
      batches      list of (b0, b1)
    per core:
      idx  [128, ntiles*8] int16  wrapped packed-row gather indices
      dst  [128, ntiles]   bf16   per-slot dst_shift in window (-1 = pad)
    """
    src = np.asarray(edge_src).astype(np.int64)
    dst = np.asarray(edge_dst).astype(np.int64)

    core = dst // SHARD
    dst_loc = dst % SHARD
    blk = dst_loc // 128
    dib = dst_loc % 128
    prow = (src // SHARD) * SPAD + (src % SHARD)
    row = prow // 4
    q = prow % 4

    key = ((core * NBLK + blk) * 4 + q)
    order = np.argsort(key, kind="stable")
    s_key, s_dib, s_row = key[order], dib[order], row[order]

    n_cells = N_CORES * NBLK * 4
    bounds = np.searchsorted(s_key, np.arange(n_cells + 1))

    ntiles = 0
    lo_of, q_of = [], []
    toff = np.zeros(NBLK + 1, np.int64)
    idx_cols = [[] for _ in range(N_CORES)]
    sh_cols = [[] for _ in range(N_CORES)]
    # pad slots must NOT all hit one table row: a hot row serializes its
    # DMA reads on one channel (measured 94 vs 208 GB/s). dsh=-1 already
    # kills pad contributions, so spread pad reads uniformly.
    prng = np.random.default_rng(12345)

    for b in range(NBLK):
        for kq in range(4):
            for h in range(1):
                segs = []
                for c in range(N_CORES):
                    cid = (c * NBLK + b) * 4 + kq
                    segs.append((int(bounds[cid]), int(bounds[cid + 1])))
                nt = (max(e - s for s, e in segs) + 127) // 128
                lo = 0
                for t in range(nt):
                    ntiles += 1
                    lo_of.append(lo)
                    q_of.append(kq)
                    for c in range(N_CORES):
                        s, e = segs[c]
                        p = s + t * 128
                        take = max(0, min(e - p, 128))
                        col_i = prng.integers(0, TROWS, 128).astype(np.int16)
                        col_s = np.full(128, -1.0, np.float32)
                        if take > 0:
                            col_i[:take] = s_row[p:p + take]
                            col_s[:take] = s_dib[p:p + take] - lo
                        idx_cols[c].append(col_i)
                        sh_cols[c].append(col_s)
        toff[b + 1] = ntiles

    batches = [(b0, min(b0 + BB, NBLK)) for b0 in range(0, NBLK, BB)]
    meta = {"ntiles": ntiles, "lo_of": lo_of, "q_of": q_of, "toff": toff,
            "batches": batches}

    per_core = []
    for c in range(N_CORES):
        icols = np.stack(idx_cols[c], 0)          # [nt, 128]
        scols = np.stack(sh_cols[c], 0)           # [nt, 128]
        w = icols.reshape(ntiles, 8, 16).transpose(2, 0, 1).reshape(16, ntiles * 8)
        per_core.append({
            "idx": np.tile(w.astype(np.int16), (8, 1)),
            "dst": scols.T.astype(ml_dtypes.bfloat16),
        })
    return meta, per_core


def _phasea_perm():
    """Phase-A node processing order: tile t covers packed rows [32t, 32t+32);
    partition p holds local node 4*(32t + p%32) + p//32."""
    t = np.arange(SPAD) // 128
    p = np.arange(SPAD) % 128
    return 4 * (32 * t + p % 32) + p // 32


def _build(meta, mode="full", n_devices=N_CORES, no_cc=False, reps=1):
    ntiles = meta["ntiles"]
    lo_of = meta["lo_of"]
    q_of = meta["q_of"]
    toff = meta["toff"]
    batches = meta["batches"]

    nc = bacc.Bacc("TRN2", target_bir_lowering=False, debug=False,
                   enable_asserts=True, num_devices=n_devices,
                   num_swdge_queues=4)

    xT = nc.dram_tensor("xT", [128, SPAD], F32, kind="ExternalInput")
    wT = nc.dram_tensor("wT", [128, OUT_DIM], F32, kind="ExternalInput")
    brep = nc.dram_tensor("brep", [128, OUT_DIM], F32, kind="ExternalInput")
    cjT = nc.dram_tensor("cjT", [128, NBLK], F32, kind="ExternalInput")
    ciT = nc.dram_tensor("ciT", [128, NBLK], F32, kind="ExternalInput")
    idx_d = nc.dram_tensor("idx", [128, ntiles * 8], I16, kind="ExternalInput")
    dst_d = nc.dram_tensor("dst", [128, ntiles], BF16, kind="ExternalInput")
    out = nc.dram_tensor("out", [SPAD, OUT_DIM], F32, kind="ExternalOutput")

    gmax = 1
    for (b0, b1) in batches:
        gmax = max(gmax, int(toff[b1] - toff[b0]))

    with tile.TileContext(nc) as tc:
        with (
            tc.tile_pool(name="dram", bufs=1, space="DRAM") as dram,
            tc.tile_pool(name="const", bufs=1) as cpool,
            tc.tile_pool(name="xa", bufs=3) as xpool,
            tc.tile_pool(name="ha", bufs=3) as hpool,
            tc.tile_pool(name="wa", bufs=3) as wpool,
            tc.tile_pool(name="pa", bufs=4, space="PSUM") as ppa,
            tc.tile_pool(name="gath", bufs=2) as gpool,
            tc.tile_pool(name="smat", bufs=2) as spool,
            tc.tile_pool(name="pb", bufs=4, space="PSUM") as ppb,
            tc.tile_pool(name="res", bufs=4) as rpool,
        ):
            table_loc = dram.tile([LROWS, ROWELEM], BF16)
            table_full = dram.tile([TROWS, ROWELEM], BF16)

            # constants
            wt_t = cpool.tile([128, OUT_DIM], F32)
            nc.sync.dma_start(out=wt_t[:], in_=wT[:])
            br_t = cpool.tile([128, OUT_DIM], F32)
            nc.sync.dma_start(out=br_t[:], in_=brep[:])
            cj_t = cpool.tile([128, NBLK], F32)
            nc.sync.dma_start(out=cj_t[:], in_=cjT[:])
            ci_t = cpool.tile([128, NBLK], F32)
            nc.sync.dma_start(out=ci_t[:], in_=ciT[:])
            idx_t = cpool.tile([128, ntiles * 8], I16)
            nc.sync.dma_start(out=idx_t[:], in_=idx_d[:])
            dst_t = cpool.tile([128, ntiles], BF16)
            nc.sync.dma_start(out=dst_t[:], in_=dst_d[:])
            # iota: [128, GRP*WIN] bf16, value = col % WIN
            io_i = cpool.tile([128, GRP * WIN], I16)
            nc.gpsimd.iota(io_i[:], pattern=[[0, GRP], [1, WIN]], base=0,
                           channel_multiplier=0)
            io_b = cpool.tile([128, GRP * WIN], BF16)
            nc.vector.tensor_copy(out=io_b[:], in_=io_i[:])
            z128 = cpool.tile([128, 128], BF16)
            nc.vector.memset(z128[:], 0)
            z32 = cpool.tile([128, OUT_DIM], BF16)
            nc.vector.memset(z32[:], 0)

            # packed-table write AP: (q, r, t, f) view of [LROWS, 128]
            tab_v = table_loc[:].rearrange("(t r) (q f) -> q r t f", r=32, q=4)

            for _rep in range(reps):
                # ---- Phase A: wh = (X @ W.T + b) * cj -> packed bf16 shard ----
                ntile_a = SPAD // 128  # 98
                for a0 in range(0, ntile_a, 4):
                    an = min(4, ntile_a - a0)
                    xt = xpool.tile([128, 4 * 128], F32)
                    nc.sync.dma_start(out=xt[:, 0:an * 128],
                                      in_=xT[:, a0 * 128:(a0 + an) * 128])
                    wh4 = wpool.tile([128, 4, OUT_DIM], BF16)
                    for j in range(an):
                        ph = ppa.tile([128, OUT_DIM], F32, space="PSUM")
                        nc.tensor.matmul(out=ph[:], lhsT=xt[:, j * 128:(j + 1) * 128],
                                         rhs=wt_t[:], start=True, stop=True)
                        hb = hpool.tile([128, OUT_DIM], F32)
                        nc.vector.tensor_add(out=hb[:], in0=ph[:], in1=br_t[:])
                        nc.vector.tensor_scalar_mul(wh4[:, j, :], hb[:],
                                                    cj_t[:, a0 + j:a0 + j + 1])
                    for j in range(an):
                        nc.sync.dma_start(out=tab_v[:, :, a0 + j, :],
                                          in_=wh4[:, j, :])

                # ---- AllGather compact table shards ----
                if mode != "A" and not no_cc:
                    nc.gpsimd.collective_compute(
                        "AllGather",
                        mybir.AluOpType.bypass,
                        replica_groups=[list(range(N_CORES))],
                        ins=[table_loc.opt()],
                        outs=[table_full.opt()],
                    )

                # ---- Phase B ----
                for (b0, b1) in batches:
                    t0, t1 = int(toff[b0]), int(toff[b1])
                    tcnt = t1 - t0
                    g = gpool.tile([128, gmax, ROWELEM], BF16, tag="g")
                    s = spool.tile([128, gmax * WIN], BF16, tag="s")
                    if tcnt > 0 and mode not in ("A", "AG"):
                        for c0 in range(0, tcnt, GCAP):
                            cn = min(GCAP, tcnt - c0)
                            nc.gpsimd.dma_gather(
                                out_ap=g[:, c0:c0 + cn, :],
                                in_ap=table_full[:],
                                idxs_ap=idx_t[:, (t0 + c0) * 8:(t0 + c0 + cn) * 8],
                                num_idxs=cn * 128,
                                num_idxs_reg=cn * 128,
                                elem_size=ROWELEM,
                                single_packet=False,
                                queue_num=(c0 // GCAP) % 4,
                            )
                        if mode != "G":
                            for g0 in range(0, tcnt, GRP):
                                cnt = min(GRP, tcnt - g0)
                                nc.vector.tensor_tensor(
                                    out=s[:, g0 * WIN:(g0 + cnt) * WIN],
                                    in0=dst_t[:, t0 + g0:t0 + g0 + cnt, None]
                                        .to_broadcast([128, cnt, WIN]),
                                    in1=io_b[:, 0:cnt * WIN],
                                    op=mybir.AluOpType.is_equal,
                                )

                    for b in range(b0, b1):
                        acc = ppb.tile([128, OUT_DIM], F32, space="PSUM")
                        nc.tensor.matmul(out=acc[:], lhsT=z128[:], rhs=z32[:],
                                         start=True, stop=(mode != "full"),
                                         skip_group_check=True)
                        if mode == "full":
                            tb0, tb1 = int(toff[b]), int(toff[b + 1])
                            for t in range(tb0, tb1):
                                gi = t - t0
                                lo = lo_of[t]
                                kq = q_of[t]
                                nc.tensor.matmul(
                                    out=acc[lo:lo + WIN, :],
                                    lhsT=s[:, gi * WIN:(gi + 1) * WIN],
                                    rhs=g[:, gi, 32 * kq:32 * kq + OUT_DIM],
                                    start=False, stop=(t == tb1 - 1),
                                    skip_group_check=True,
                                )
                        res = rpool.tile([128, OUT_DIM], F32)
                        nc.vector.tensor_scalar_mul(res[:], acc[:], ci_t[:, b:b + 1])
                        nc.sync.dma_start(out=out[b * 128:(b + 1) * 128, :], in_=res[:])
    nc.compile()
    return nc


def _in_maps(ins, per_core):
    src_feats = np.ascontiguousarray(np.asarray(ins["src_feats"], dtype=np.float32))
    cj = np.asarray(ins["cj"], dtype=np.float32).reshape(-1)
    ci = np.asarray(ins["ci"], dtype=np.float32).reshape(-1)
    W = np.asarray(ins["W"], dtype=np.float32)
    b = np.asarray(ins["b"], dtype=np.float32).reshape(-1)

    perm = _phasea_perm()
    maps = []
    for c in range(N_CORES):
        lo, hi = c * SHARD, (c + 1) * SHARD
        xf = np.zeros((SPAD, IN_DIM), np.float32)
        xf[:SHARD] = src_feats[lo:hi]
        cjf = np.zeros(SPAD, np.float32)
        cjf[:SHARD] = cj[lo:hi]
        cif = np.zeros(SPAD, np.float32)
        cif[:SHARD] = ci[lo:hi]
        xP = xf[perm]            # phase-A processing order
        cjP = cjf[perm]
        m = {
            "xT": np.ascontiguousarray(xP.T),
            "wT": np.ascontiguousarray(W.T),
            "brep": np.tile(b[None, :], (128, 1)),
            "cjT": np.ascontiguousarray(cjP.reshape(NBLK, 128).T),
            "ciT": np.ascontiguousarray(cif.reshape(NBLK, 128).T),
        }
        m.update(per_core[c])
        maps.append(m)
    return maps


def kernel(src_feats, cj, ci, W, b, edge_src, edge_dst):
    ins = {"src_feats": src_feats, "cj": cj, "ci": ci, "W": W, "b": b}
    meta, per_core = _plan(edge_src, edge_dst)
    nc = _build(meta)
    maps = _in_maps(ins, per_core)
    res = run_bass_kernel_spmd(nc, maps, core_ids=list(range(N_CORES)))
    outs = [res.results[c]["out"][:SHARD] for c in range(N_CORES)]
    return np.concatenate(outs, 0).astype(np.float32)



# revision 13
# speedup vs baseline: 3.8598x; 1.0152x over previous
"""GCMC graph-conv kernel for Trainium2, distributed over 8 NeuronCores.

Computes: agg = segment_sum((src_feats @ W.T + b) * cj [edge_src], edge_dst) * ci

Strategy (dst-sharded, one NEFF SPMD on 8 cores):
  - Each core owns 12500 destination nodes and the edges pointing to them.
  - Phase A: each core computes wh = (X_shard @ W.T + b) * cj_shard on the
    TensorEngine and writes it (bf16) into a packed table shard: each 256B row
    holds FOUR nodes' 32-feature messages (node prow -> row prow//4, subcol
    prow%4). Packing keeps dma_gather rows at the required 256B multiple while
    the whole 25088-row table stays addressable by int16 gather indices.
  - AllGather the 8 compact shards (0.8MB each) -> full table in every HBM.
  - Phase B: edges are bucketed by (dst block, q=prow%4, dst half). Each
    128-slot tile gathers its edges' table rows (dma_gather), builds a
    one-hot matrix over its 64-dst window (is_equal on VectorE), and
    scatter-sums via PSUM-accumulating matmuls (rhs = gathered columns
    [32q : 32q+32]). Scale by ci, DMA out.

All control structure (tile counts, windows) is common across the 8 cores
(max over cores); cores pad their slots (dst_shift=-1 kills the one-hot
column; gather idx 0 is harmless).
"""
import sys

if "/opt/trn_rl_repo" not in sys.path:
    sys.path.insert(0, "/opt/trn_rl_repo")

import numpy as np
import ml_dtypes

import concourse.bacc as bacc
import concourse.mybir as mybir
import concourse.tile as tile
from concourse.bass_utils import run_bass_kernel_spmd

# problem constants (hardcoded per harness contract)
N_NODES = 100000
N_EDGES = 1_600_000
IN_DIM = 128
OUT_DIM = 32
N_CORES = 8
SHARD = N_NODES // N_CORES          # 12500 dst nodes per core
NBLK = (SHARD + 127) // 128         # 98 dst blocks per core
SPAD = NBLK * 128                   # 12544 padded shard nodes
TROWS = SPAD * N_CORES // 4         # 25088 packed table rows (4 nodes each)
LROWS = SPAD // 4                   # 3136 packed rows per core shard
HLF = LROWS // 2                    # 1568 rows per collective half
ROWELEM = 128                       # bf16 elems per table row = 256B
WIN = 128                           # one-hot window: full block (PSUM base 0)
GRP = 8                             # tiles per is_equal op
BB = 5                              # dst blocks per double-buffered batch
GCAP = 25                           # tiles per dma_gather call

F32 = mybir.dt.float32
BF16 = mybir.dt.bfloat16
I16 = mybir.dt.int16


def _plan(edge_src, edge_dst):
    """Pack edges into the common SPMD structure.

    meta:
      ntiles       total tiles
      lo_of[t]     PSUM window base (0 or 64)
      q_of[t]      table subcolumn (edge prow % 4)
      toff[b]      first tile of dst block b
      batches      list of (b0, b1)
    per core:
      idx  [128, ntiles*8] int16  wrapped packed-row gather indices
      dst  [128, ntiles]   bf16   per-slot dst_shift in window (-1 = pad)
    """
    src = np.asarray(edge_src).astype(np.int64)
    dst = np.asarray(edge_dst).astype(np.int64)

    core = dst // SHARD
    dst_loc = dst % SHARD
    blk = dst_loc // 128
    dib = dst_loc % 128
    prow = (src // SHARD) * SPAD + (src % SHARD)
    # table_full is laid out half-major so the AllGather can run as two
    # pipelined collectives overlapped with phase A:
    #   nrow = half*(8*HLF) + src_core*HLF + (local_packed_row % HLF)
    l = (prow % SPAD) // 4
    row = (l // HLF) * (N_CORES * HLF) + (src // SHARD) * HLF + (l % HLF)
    q = prow % 4

    key = ((core * NBLK + blk) * 4 + q)
    order = np.argsort(key, kind="stable")
    s_key, s_dib, s_row = key[order], dib[order], row[order]

    n_cells = N_CORES * NBLK * 4
    bounds = np.searchsorted(s_key, np.arange(n_cells + 1))

    ntiles = 0
    lo_of, q_of = [], []
    toff = np.zeros(NBLK + 1, np.int64)
    idx_cols = [[] for _ in range(N_CORES)]
    sh_cols = [[] for _ in range(N_CORES)]
    # pad slots must NOT all hit one table row: a hot row serializes its
    # DMA reads on one channel (measured 94 vs 208 GB/s). dsh=-1 already
    # kills pad contributions, so spread pad reads uniformly.
    prng = np.random.default_rng(12345)

    for b in range(NBLK):
        for kq in range(4):
            for h in range(1):
                segs = []
                for c in range(N_CORES):
                    cid = (c * NBLK + b) * 4 + kq
                    segs.append((int(bounds[cid]), int(bounds[cid + 1])))
                nt = (max(e - s for s, e in segs) + 127) // 128
                lo = 0
                for t in range(nt):
                    ntiles += 1
                    lo_of.append(lo)
                    q_of.append(kq)
                    for c in range(N_CORES):
                        s, e = segs[c]
                        p = s + t * 128
                        take = max(0, min(e - p, 128))
                        col_i = prng.integers(0, TROWS, 128).astype(np.int16)
                        col_s = np.full(128, -1.0, np.float32)
                        if take > 0:
                            col_i[:take] = s_row[p:p + take]
                            col_s[:take] = s_dib[p:p + take] - lo
                        idx_cols[c].append(col_i)
                        sh_cols[c].append(col_s)
        toff[b + 1] = ntiles

    batches = [(b0, min(b0 + BB, NBLK)) for b0 in range(0, NBLK, BB)]
    meta = {"ntiles": ntiles, "lo_of": lo_of, "q_of": q_of, "toff": toff,
            "batches": batches}

    per_core = []
    for c in range(N_CORES):
        icols = np.stack(idx_cols[c], 0)          # [nt, 128]
        scols = np.stack(sh_cols[c], 0)           # [nt, 128]
        w = icols.reshape(ntiles, 8, 16).transpose(2, 0, 1).reshape(16, ntiles * 8)
        per_core.append({
            "idx": np.tile(w.astype(np.int16), (8, 1)),
            "dst": scols.T.astype(ml_dtypes.bfloat16),
        })
    return meta, per_core


def _phasea_perm():
    """Phase-A node order: with partition p = (row_in_tile*4 + subcol), the
    node at (tile t, partition p) is simply 128t + p — identity order."""
    return np.arange(SPAD)


def _build(meta, mode="full", n_devices=N_CORES, no_cc=False, reps=1):
    ntiles = meta["ntiles"]
    lo_of = meta["lo_of"]
    q_of = meta["q_of"]
    toff = meta["toff"]
    batches = meta["batches"]

    nc = bacc.Bacc("TRN2", target_bir_lowering=False, debug=False,
                   enable_asserts=True, num_devices=n_devices,
                   num_swdge_queues=4)

    xT = nc.dram_tensor("xT", [128, SPAD], F32, kind="ExternalInput")
    wT = nc.dram_tensor("wT", [128, OUT_DIM], F32, kind="ExternalInput")
    brep = nc.dram_tensor("brep", [128, OUT_DIM], F32, kind="ExternalInput")
    cjT = nc.dram_tensor("cjT", [128, NBLK], F32, kind="ExternalInput")
    ciT = nc.dram_tensor("ciT", [128, NBLK], F32, kind="ExternalInput")
    idx_d = nc.dram_tensor("idx", [128, ntiles * 8], I16, kind="ExternalInput")
    dst_d = nc.dram_tensor("dst", [128, ntiles], BF16, kind="ExternalInput")
    out = nc.dram_tensor("out", [SPAD, OUT_DIM], F32, kind="ExternalOutput")

    gmax = 1
    for (b0, b1) in batches:
        gmax = max(gmax, int(toff[b1] - toff[b0]))

    with tile.TileContext(nc) as tc:
        with (
            tc.tile_pool(name="dram", bufs=1, space="DRAM") as dram,
            tc.tile_pool(name="const", bufs=1) as cpool,
            tc.tile_pool(name="xa", bufs=2) as xpool,
            tc.tile_pool(name="ha", bufs=3) as hpool,
            tc.tile_pool(name="wa", bufs=3) as wpool,
            tc.tile_pool(name="pa", bufs=4, space="PSUM") as ppa,
            tc.tile_pool(name="gath", bufs=2) as gpool,
            tc.tile_pool(name="smat", bufs=2) as spool,
            tc.tile_pool(name="pb", bufs=4, space="PSUM") as ppb,
            tc.tile_pool(name="res", bufs=4) as rpool,
        ):
            table_loc = dram.tile([LROWS, ROWELEM], BF16)
            table_full = dram.tile([TROWS, ROWELEM], BF16)

            # constants
            wt_t = cpool.tile([128, OUT_DIM], F32)
            nc.sync.dma_start(out=wt_t[:], in_=wT[:])
            br_t = cpool.tile([128, OUT_DIM], F32)
            nc.sync.dma_start(out=br_t[:], in_=brep[:])
            cj_t = cpool.tile([128, NBLK], F32)
            nc.sync.dma_start(out=cj_t[:], in_=cjT[:])
            ci_t = cpool.tile([128, NBLK], F32)
            nc.sync.dma_start(out=ci_t[:], in_=ciT[:])
            idx_t = cpool.tile([128, ntiles * 8], I16)
            nc.sync.dma_start(out=idx_t[:], in_=idx_d[:])
            dst_t = cpool.tile([128, ntiles], BF16)
            nc.sync.dma_start(out=dst_t[:], in_=dst_d[:])
            # iota: [128, GRP*WIN] bf16, value = col % WIN
            io_i = cpool.tile([128, GRP * WIN], I16)
            nc.gpsimd.iota(io_i[:], pattern=[[0, GRP], [1, WIN]], base=0,
                           channel_multiplier=0)
            io_b = cpool.tile([128, GRP * WIN], BF16)
            nc.vector.tensor_copy(out=io_b[:], in_=io_i[:])
            z128 = cpool.tile([128, 128], BF16)
            nc.vector.memset(z128[:], 0)
            z32 = cpool.tile([128, OUT_DIM], BF16)
            nc.vector.memset(z32[:], 0)

            # packed-table write AP: partition p = (row_in_tile*4 + subcol),
            # so node (128t + p) lives at table row 32t+p//4, subcol p%4 and
            # the per-partition DRAM stride is a uniform 64B (3-dim AP).
            tab_v = table_loc[:].rearrange("(t a) (b f) -> (a b) t f", a=32, b=4)

            for _rep in range(reps):
                # ---- Phase A: wh = (X @ W.T + b) * cj -> packed bf16 shard,
                # in two halves; each half's AllGather overlaps the other
                # half's compute ----
                HB = SPAD // 128 // 2  # 49 blocks per half
                for h in range(2):
                    a0 = h * HB
                    xt = xpool.tile([128, HB * 128], F32)
                    nc.sync.dma_start(out=xt[:],
                                      in_=xT[:, a0 * 128:(a0 + HB) * 128])
                    whh = wpool.tile([128, HB, OUT_DIM], BF16)
                    for j in range(HB):
                        ph = ppa.tile([128, OUT_DIM], F32, space="PSUM")
                        nc.tensor.matmul(out=ph[:], lhsT=xt[:, j * 128:(j + 1) * 128],
                                         rhs=wt_t[:], start=True, stop=True)
                        hb = hpool.tile([128, OUT_DIM], F32)
                        nc.vector.tensor_add(out=hb[:], in0=ph[:], in1=br_t[:])
                        nc.vector.tensor_scalar_mul(whh[:, j, :], hb[:],
                                                    cj_t[:, a0 + j:a0 + j + 1])
                    nc.sync.dma_start(out=tab_v[:, a0:a0 + HB, :],
                                      in_=whh[:])
                    if mode != "A" and not no_cc:
                        nc.gpsimd.collective_compute(
                            "AllGather",
                            mybir.AluOpType.bypass,
                            replica_groups=[list(range(N_CORES))],
                            ins=[table_loc[h * HLF:(h + 1) * HLF, :].opt()],
                            outs=[table_full[h * 8 * HLF:(h + 1) * 8 * HLF, :].opt()],
                        )

                # ---- Phase B ----
                for (b0, b1) in batches:
                    t0, t1 = int(toff[b0]), int(toff[b1])
                    tcnt = t1 - t0
                    g = gpool.tile([128, gmax, ROWELEM], BF16, tag="g")
                    s = spool.tile([128, gmax * WIN], BF16, tag="s")
                    if tcnt > 0 and mode not in ("A", "AG"):
                        for c0 in range(0, tcnt, GCAP):
                            cn = min(GCAP, tcnt - c0)
                            nc.gpsimd.dma_gather(
                                out_ap=g[:, c0:c0 + cn, :],
                                in_ap=table_full[:],
                                idxs_ap=idx_t[:, (t0 + c0) * 8:(t0 + c0 + cn) * 8],
                                num_idxs=cn * 128,
                                num_idxs_reg=cn * 128,
                                elem_size=ROWELEM,
                                single_packet=False,
                                queue_num=(c0 // GCAP) % 4,
                            )
                        if mode != "G":
                            for g0 in range(0, tcnt, GRP):
                                cnt = min(GRP, tcnt - g0)
                                nc.vector.tensor_tensor(
                                    out=s[:, g0 * WIN:(g0 + cnt) * WIN],
                                    in0=dst_t[:, t0 + g0:t0 + g0 + cnt, None]
                                        .to_broadcast([128, cnt, WIN]),
                                    in1=io_b[:, 0:cnt * WIN],
                                    op=mybir.AluOpType.is_equal,
                                )

                    for b in range(b0, b1):
                        acc = ppb.tile([128, OUT_DIM], F32, space="PSUM")
                        nc.tensor.matmul(out=acc[:], lhsT=z128[:], rhs=z32[:],
                                         start=True, stop=(mode != "full"),
                                         skip_group_check=True)
                        if mode == "full":
                            tb0, tb1 = int(toff[b]), int(toff[b + 1])
                            for t in range(tb0, tb1):
                                gi = t - t0
                                lo = lo_of[t]
                                kq = q_of[t]
                                nc.tensor.matmul(
                                    out=acc[lo:lo + WIN, :],
                                    lhsT=s[:, gi * WIN:(gi + 1) * WIN],
                                    rhs=g[:, gi, 32 * kq:32 * kq + OUT_DIM],
                                    start=False, stop=(t == tb1 - 1),
                                    skip_group_check=True,
                                )
                        res = rpool.tile([128, OUT_DIM], F32)
                        nc.vector.tensor_scalar_mul(res[:], acc[:], ci_t[:, b:b + 1])
                        nc.sync.dma_start(out=out[b * 128:(b + 1) * 128, :], in_=res[:])
    nc.compile()
    return nc


def _in_maps(ins, per_core):
    src_feats = np.ascontiguousarray(np.asarray(ins["src_feats"], dtype=np.float32))
    cj = np.asarray(ins["cj"], dtype=np.float32).reshape(-1)
    ci = np.asarray(ins["ci"], dtype=np.float32).reshape(-1)
    W = np.asarray(ins["W"], dtype=np.float32)
    b = np.asarray(ins["b"], dtype=np.float32).reshape(-1)

    perm = _phasea_perm()
    maps = []
    for c in range(N_CORES):
        lo, hi = c * SHARD, (c + 1) * SHARD
        xf = np.zeros((SPAD, IN_DIM), np.float32)
        xf[:SHARD] = src_feats[lo:hi]
        cjf = np.zeros(SPAD, np.float32)
        cjf[:SHARD] = cj[lo:hi]
        cif = np.zeros(SPAD, np.float32)
        cif[:SHARD] = ci[lo:hi]
        xP = xf[perm]            # phase-A processing order
        cjP = cjf[perm]
        m = {
            "xT": np.ascontiguousarray(xP.T),
            "wT": np.ascontiguousarray(W.T),
            "brep": np.tile(b[None, :], (128, 1)),
            "cjT": np.ascontiguousarray(cjP.reshape(NBLK, 128).T),
            "ciT": np.ascontiguousarray(cif.reshape(NBLK, 128).T),
        }
        m.update(per_core[c])
        maps.append(m)
    return maps


def kernel(src_feats, cj, ci, W, b, edge_src, edge_dst):
    ins = {"src_feats": src_feats, "cj": cj, "ci": ci, "W": W, "b": b}
    meta, per_core = _plan(edge_src, edge_dst)
    nc = _build(meta)
    maps = _in_maps(ins, per_core)
    res = run_bass_kernel_spmd(nc, maps, core_ids=list(range(N_CORES)))
    outs = [res.results[c]["out"][:SHARD] for c in range(N_CORES)]
    return np.concatenate(outs, 0).astype(np.float32)



# revision 16
# speedup vs baseline: 4.7179x; 1.2223x over previous
"""GCMC graph-conv kernel for Trainium2, distributed over 8 NeuronCores.

Computes: agg = segment_sum((src_feats @ W.T + b) * cj [edge_src], edge_dst) * ci

Strategy (dst-sharded, one NEFF SPMD on 8 cores):
  - Each core owns 12500 destination nodes and the edges pointing to them.
  - Phase A: each core computes wh = (X_shard @ W.T + b) * cj_shard on the
    TensorEngine and writes it (bf16) into a packed table shard: each 256B row
    holds FOUR nodes' 32-feature messages (node prow -> row prow//4, subcol
    prow%4). Packing keeps dma_gather rows at the required 256B multiple while
    the whole 25088-row table stays addressable by int16 gather indices.
  - AllGather the 8 compact shards (0.8MB each) -> full table in every HBM.
  - Phase B: edges are bucketed by (dst block, q=prow%4, dst half). Each
    128-slot tile gathers its edges' table rows (dma_gather), builds a
    one-hot matrix over its 64-dst window (is_equal on VectorE), and
    scatter-sums via PSUM-accumulating matmuls (rhs = gathered columns
    [32q : 32q+32]). Scale by ci, DMA out.

All control structure (tile counts, windows) is common across the 8 cores
(max over cores); cores pad their slots (dst_shift=-1 kills the one-hot
column; gather idx 0 is harmless).
"""
import sys

if "/opt/trn_rl_repo" not in sys.path:
    sys.path.insert(0, "/opt/trn_rl_repo")

import numpy as np
import ml_dtypes

import concourse.bacc as bacc
import concourse.mybir as mybir
import concourse.tile as tile
from concourse.bass_utils import run_bass_kernel_spmd

# problem constants (hardcoded per harness contract)
N_NODES = 100000
N_EDGES = 1_600_000
IN_DIM = 128
OUT_DIM = 32
N_CORES = 8
SHARD = N_NODES // N_CORES          # 12500 dst nodes per core
NBLK = (SHARD + 127) // 128         # 98 dst blocks per core
SPAD = NBLK * 128                   # 12544 padded shard nodes
TROWS = SPAD * N_CORES // 4         # 25088 packed table rows (4 nodes each)
LROWS = SPAD // 4                   # 3136 packed rows per core shard
HLF = LROWS // 2                    # 1568 rows per collective half
ROWELEM = 128                       # bf16 elems per table row = 256B
WIN = 128                           # one-hot window: full block (PSUM base 0)
GRP = 8                             # tiles per is_equal op
BB = 5                              # dst blocks per double-buffered batch
GCAP = 25                           # tiles per dma_gather call

F32 = mybir.dt.float32
BF16 = mybir.dt.bfloat16
I16 = mybir.dt.int16


def _plan(edge_src, edge_dst):
    """Pack edges into the common SPMD structure.

    meta:
      ntiles       total tiles
      lo_of[t]     PSUM window base (0 or 64)
      q_of[t]      table subcolumn (edge prow % 4)
      toff[b]      first tile of dst block b
      batches      list of (b0, b1)
    per core:
      idx  [128, ntiles*8] int16  wrapped packed-row gather indices
      dst  [128, ntiles]   bf16   per-slot dst_shift in window (-1 = pad)
    """
    src = np.asarray(edge_src).astype(np.int64)
    dst = np.asarray(edge_dst).astype(np.int64)

    core = dst // SHARD
    dst_loc = dst % SHARD
    blk = dst_loc // 128
    dib = dst_loc % 128
    prow = (src // SHARD) * SPAD + (src % SHARD)
    # table_full is laid out half-major so the AllGather can run as two
    # pipelined collectives overlapped with phase A:
    #   nrow = half*(8*HLF) + src_core*HLF + (local_packed_row % HLF)
    l = (prow % SPAD) // 4
    row = (l // HLF) * (N_CORES * HLF) + (src // SHARD) * HLF + (l % HLF)
    q = prow % 4

    key = ((core * NBLK + blk) * 4 + q)
    order = np.argsort(key, kind="stable")
    s_key, s_dib, s_row = key[order], dib[order], row[order]

    n_cells = N_CORES * NBLK * 4
    bounds = np.searchsorted(s_key, np.arange(n_cells + 1))

    ntiles = 0
    lo_of, q_of = [], []
    toff = np.zeros(NBLK + 1, np.int64)
    idx_cols = [[] for _ in range(N_CORES)]
    sh_cols = [[] for _ in range(N_CORES)]
    # pad slots must NOT all hit one table row: a hot row serializes its
    # DMA reads on one channel (measured 94 vs 208 GB/s). dsh=-1 already
    # kills pad contributions, so spread pad reads uniformly.
    prng = np.random.default_rng(12345)

    for b in range(NBLK):
        for kq in range(4):
            for h in range(1):
                segs = []
                for c in range(N_CORES):
                    cid = (c * NBLK + b) * 4 + kq
                    segs.append((int(bounds[cid]), int(bounds[cid + 1])))
                nt = (max(e - s for s, e in segs) + 127) // 128
                lo = 0
                for t in range(nt):
                    ntiles += 1
                    lo_of.append(lo)
                    q_of.append(kq)
                    for c in range(N_CORES):
                        s, e = segs[c]
                        p = s + t * 128
                        take = max(0, min(e - p, 128))
                        col_i = prng.integers(0, TROWS, 128).astype(np.int16)
                        col_s = np.full(128, -1.0, np.float32)
                        if take > 0:
                            col_i[:take] = s_row[p:p + take]
                            col_s[:take] = s_dib[p:p + take] - lo
                        idx_cols[c].append(col_i)
                        sh_cols[c].append(col_s)
        toff[b + 1] = ntiles

    batches = [(b0, min(b0 + BB, NBLK)) for b0 in range(0, NBLK, BB)]
    meta = {"ntiles": ntiles, "lo_of": lo_of, "q_of": q_of, "toff": toff,
            "batches": batches}

    per_core = []
    for c in range(N_CORES):
        icols = np.stack(idx_cols[c], 0)          # [nt, 128]
        scols = np.stack(sh_cols[c], 0)           # [nt, 128]
        w = icols.reshape(ntiles, 8, 16).transpose(2, 0, 1).reshape(16, ntiles * 8)
        per_core.append({
            "idx": np.tile(w.astype(np.int16), (8, 1)),
            "dst": scols.T.astype(ml_dtypes.bfloat16),
        })
    return meta, per_core


def _phasea_perm():
    """Phase-A node order: with partition p = (row_in_tile*4 + subcol), the
    node at (tile t, partition p) is simply 128t + p — identity order."""
    return np.arange(SPAD)


def _build(meta, mode="full", n_devices=N_CORES, no_cc=False, reps=1):
    ntiles = meta["ntiles"]
    lo_of = meta["lo_of"]
    q_of = meta["q_of"]
    toff = meta["toff"]
    batches = meta["batches"]

    nc = bacc.Bacc("TRN2", target_bir_lowering=False, debug=False,
                   enable_asserts=True, num_devices=n_devices,
                   num_swdge_queues=4)

    xT = nc.dram_tensor("xT", [128, SPAD], F32, kind="ExternalInput")
    wT = nc.dram_tensor("wT", [128, OUT_DIM], F32, kind="ExternalInput")
    brep = nc.dram_tensor("brep", [128, OUT_DIM], F32, kind="ExternalInput")
    cjT = nc.dram_tensor("cjT", [128, NBLK], F32, kind="ExternalInput")
    ciT = nc.dram_tensor("ciT", [128, NBLK], F32, kind="ExternalInput")
    idx_d = nc.dram_tensor("idx", [128, ntiles * 8], I16, kind="ExternalInput")
    dst_d = nc.dram_tensor("dst", [128, ntiles], BF16, kind="ExternalInput")
    out = nc.dram_tensor("out", [SPAD, OUT_DIM], F32, kind="ExternalOutput")

    gmax = 1
    for (b0, b1) in batches:
        gmax = max(gmax, int(toff[b1] - toff[b0]))

    with tile.TileContext(nc) as tc:
        with (
            tc.tile_pool(name="dram", bufs=1, space="DRAM") as dram,
            tc.tile_pool(name="const", bufs=1) as cpool,
            tc.tile_pool(name="xa", bufs=1) as xpool,
            tc.tile_pool(name="ha", bufs=3) as hpool,
            tc.tile_pool(name="wa", bufs=3) as wpool,
            tc.tile_pool(name="pa", bufs=4, space="PSUM") as ppa,
            tc.tile_pool(name="gath", bufs=3) as gpool,
            tc.tile_pool(name="smat", bufs=2) as spool,
            tc.tile_pool(name="pb", bufs=4, space="PSUM") as ppb,
            tc.tile_pool(name="res", bufs=4) as rpool,
        ):
            table_loc = dram.tile([LROWS, ROWELEM], BF16)
            table_full = dram.tile([TROWS, ROWELEM], BF16)

            # constants
            wt_t = cpool.tile([128, OUT_DIM], F32)
            nc.sync.dma_start(out=wt_t[:], in_=wT[:])
            br_t = cpool.tile([128, OUT_DIM], F32)
            nc.sync.dma_start(out=br_t[:], in_=brep[:])
            cj_t = cpool.tile([128, NBLK], F32)
            nc.sync.dma_start(out=cj_t[:], in_=cjT[:])
            ci_t = cpool.tile([128, NBLK], F32)
            nc.sync.dma_start(out=ci_t[:], in_=ciT[:])
            idx_t = cpool.tile([128, ntiles * 8], I16)
            nc.sync.dma_start(out=idx_t[:], in_=idx_d[:])
            dst_t = cpool.tile([128, ntiles], BF16)
            nc.sync.dma_start(out=dst_t[:], in_=dst_d[:])
            # iota: [128, GRP*WIN] bf16, value = col % WIN
            io_i = cpool.tile([128, GRP * WIN], I16)
            nc.gpsimd.iota(io_i[:], pattern=[[0, GRP], [1, WIN]], base=0,
                           channel_multiplier=0)
            io_b = cpool.tile([128, GRP * WIN], BF16)
            nc.vector.tensor_copy(out=io_b[:], in_=io_i[:])
            z128 = cpool.tile([128, 128], BF16)
            nc.vector.memset(z128[:], 0)
            z32 = cpool.tile([128, OUT_DIM], BF16)
            nc.vector.memset(z32[:], 0)

            # packed-table write AP: partition p = (row_in_tile*4 + subcol),
            # so node (128t + p) lives at table row 32t+p//4, subcol p%4 and
            # the per-partition DRAM stride is a uniform 64B (3-dim AP).
            tab_v = table_loc[:].rearrange("(t a) (b f) -> (a b) t f", a=32, b=4)

            for _rep in range(reps):
                # ---- Phase A: wh = (X @ W.T + b) * cj -> packed bf16 shard,
                # in two halves; each half's AllGather overlaps the other
                # half's compute ----
                HB = SPAD // 128 // 2  # 49 blocks per half
                for h in range(2):
                    a0 = h * HB
                    xt = xpool.tile([128, HB * 128], F32)
                    nc.sync.dma_start(out=xt[:],
                                      in_=xT[:, a0 * 128:(a0 + HB) * 128])
                    whh = wpool.tile([128, HB, OUT_DIM], BF16)
                    for j in range(HB):
                        ph = ppa.tile([128, OUT_DIM], F32, space="PSUM")
                        nc.tensor.matmul(out=ph[:], lhsT=xt[:, j * 128:(j + 1) * 128],
                                         rhs=wt_t[:], start=True, stop=True)
                        hb = hpool.tile([128, OUT_DIM], F32)
                        nc.vector.tensor_add(out=hb[:], in0=ph[:], in1=br_t[:])
                        nc.vector.tensor_scalar_mul(whh[:, j, :], hb[:],
                                                    cj_t[:, a0 + j:a0 + j + 1])
                    nc.sync.dma_start(out=tab_v[:, a0:a0 + HB, :],
                                      in_=whh[:])
                    if mode != "A" and not no_cc:
                        nc.gpsimd.collective_compute(
                            "AllGather",
                            mybir.AluOpType.bypass,
                            replica_groups=[list(range(N_CORES))],
                            ins=[table_loc[h * HLF:(h + 1) * HLF, :].opt()],
                            outs=[table_full[h * 8 * HLF:(h + 1) * 8 * HLF, :].opt()],
                        )

                # ---- Phase B ----
                for (b0, b1) in batches:
                    t0, t1 = int(toff[b0]), int(toff[b1])
                    tcnt = t1 - t0
                    g = gpool.tile([128, gmax, ROWELEM], BF16, tag="g")
                    s = spool.tile([128, gmax * WIN], BF16, tag="s")
                    if tcnt > 0 and mode not in ("A", "AG"):
                        for c0 in range(0, tcnt, GCAP):
                            cn = min(GCAP, tcnt - c0)
                            nc.gpsimd.dma_gather(
                                out_ap=g[:, c0:c0 + cn, :],
                                in_ap=table_full[:],
                                idxs_ap=idx_t[:, (t0 + c0) * 8:(t0 + c0 + cn) * 8],
                                num_idxs=cn * 128,
                                num_idxs_reg=cn * 128,
                                elem_size=ROWELEM,
                                single_packet=False,
                                queue_num=(c0 // GCAP) % 4,
                            )
                        if mode != "G":
                            for g0 in range(0, tcnt, GRP):
                                cnt = min(GRP, tcnt - g0)
                                nc.vector.tensor_tensor(
                                    out=s[:, g0 * WIN:(g0 + cnt) * WIN],
                                    in0=dst_t[:, t0 + g0:t0 + g0 + cnt, None]
                                        .to_broadcast([128, cnt, WIN]),
                                    in1=io_b[:, 0:cnt * WIN],
                                    op=mybir.AluOpType.is_equal,
                                )

                    for b in range(b0, b1):
                        acc = ppb.tile([128, OUT_DIM], F32, space="PSUM")
                        if mode == "full":
                            tb0, tb1 = int(toff[b]), int(toff[b + 1])
                            for t in range(tb0, tb1):
                                gi = t - t0
                                kq = q_of[t]
                                nc.tensor.matmul(
                                    out=acc[:],
                                    lhsT=s[:, gi * WIN:(gi + 1) * WIN],
                                    rhs=g[:, gi, 32 * kq:32 * kq + OUT_DIM],
                                    start=(t == tb0), stop=(t == tb1 - 1),
                                    skip_group_check=True,
                                )
                        else:
                            nc.tensor.matmul(out=acc[:], lhsT=z128[:], rhs=z32[:],
                                             start=True, stop=True,
                                             skip_group_check=True)
                        res = rpool.tile([128, OUT_DIM], F32)
                        nc.vector.tensor_scalar_mul(res[:], acc[:], ci_t[:, b:b + 1])
                        nc.sync.dma_start(out=out[b * 128:(b + 1) * 128, :], in_=res[:])
    nc.compile()
    return nc


def _in_maps(ins, per_core):
    src_feats = np.ascontiguousarray(np.asarray(ins["src_feats"], dtype=np.float32))
    cj = np.asarray(ins["cj"], dtype=np.float32).reshape(-1)
    ci = np.asarray(ins["ci"], dtype=np.float32).reshape(-1)
    W = np.asarray(ins["W"], dtype=np.float32)
    b = np.asarray(ins["b"], dtype=np.float32).reshape(-1)

    perm = _phasea_perm()
    maps = []
    for c in range(N_CORES):
        lo, hi = c * SHARD, (c + 1) * SHARD
        xf = np.zeros((SPAD, IN_DIM), np.float32)
        xf[:SHARD] = src_feats[lo:hi]
        cjf = np.zeros(SPAD, np.float32)
        cjf[:SHARD] = cj[lo:hi]
        cif = np.zeros(SPAD, np.float32)
        cif[:SHARD] = ci[lo:hi]
        xP = xf[perm]            # phase-A processing order
        cjP = cjf[perm]
        m = {
            "xT": np.ascontiguousarray(xP.T),
            "wT": np.ascontiguousarray(W.T),
            "brep": np.tile(b[None, :], (128, 1)),
            "cjT": np.ascontiguousarray(cjP.reshape(NBLK, 128).T),
            "ciT": np.ascontiguousarray(cif.reshape(NBLK, 128).T),
        }
        m.update(per_core[c])
        maps.append(m)
    return maps


def kernel(src_feats, cj, ci, W, b, edge_src, edge_dst):
    ins = {"src_feats": src_feats, "cj": cj, "ci": ci, "W": W, "b": b}
    meta, per_core = _plan(edge_src, edge_dst)
    nc = _build(meta)
    maps = _in_maps(ins, per_core)
    res = run_bass_kernel_spmd(nc, maps, core_ids=list(range(N_CORES)))
    outs = [res.results[c]["out"][:SHARD] for c in range(N_CORES)]
    return np.concatenate(outs, 0).astype(np.float32)

